# revision 2
# baseline (speedup 1.0000x reference)
"""Grok1-style MoE (E=8 experts, top-2, H=2048, I=4096, T=8192) on 8 trn2 NeuronCores.

Strategy: expert parallelism with host-side routing.
- Host computes the (tiny: ~0.3 GFLOP of ~6.6 TFLOP total) router matmul +
  softcapped softmax + top-2 selection, gathers each expert's tokens, and
  packs per-core inputs. Core e runs expert e's FFN over its ~T*2/E tokens.
- Device kernel per core (bf16 matmuls, fp32 accumulate):
    hT  = silu(w1.T @ xT) * (w3.T @ xT)      # [I, C] in transposed layout
    outT = w2.T @ hT                          # [H, C]
  All operands are laid out on host so every DMA is a contiguous slice and
  every matmul lhsT/rhs is a natural [K=128, M/N] tile.
- Host scatter-adds `probs[t, e] * outT.T` into the full output.
"""

import os
import sys

for _p in ("/opt/trn_rl_repo", "/root/.axon_site/_ro/trn_rl_repo"):
    if os.path.isdir(_p) and _p not in sys.path:
        sys.path.insert(0, _p)

import numpy as np
import ml_dtypes

import concourse.bass as bass  # noqa: F401  (registers types)
import concourse.mybir as mybir
import concourse.tile as tile
from concourse import bacc
from concourse.bass_utils import run_bass_kernel_spmd

BF16 = mybir.dt.bfloat16
F32 = mybir.dt.float32
AF = mybir.ActivationFunctionType

E, TOPK, H, I = 8, 2, 2048, 4096
SOFTCAP = 30.0
KH = H // 128   # 16 k-tiles over H
KI = I // 128   # 32 k-tiles over I
GROUP_MAX = 1152  # max token-columns resident per group (SBUF budget)

_prog_cache: dict = {}


def _chunk_plan(C: int):
    """Split [0, C) into matmul-N chunks (<=512) packed into SBUF groups."""
    chunks = []
    c = 0
    while c < C:
        w = min(512, C - c)
        chunks.append((c, w))
        c += w
    groups = []  # (g0, gw, [(rel_off, w), ...])
    cur, cur_w = [], 0
    for off, w in chunks:
        if cur and cur_w + w > GROUP_MAX:
            groups.append((cur[0][0], cur_w, [(o - cur[0][0], ww) for o, ww in cur]))
            cur, cur_w = [], 0
        cur.append((off, w))
        cur_w += w
    if cur:
        groups.append((cur[0][0], cur_w, [(o - cur[0][0], ww) for o, ww in cur]))
    return groups


def _build_program(C: int):
    key = C
    if key in _prog_cache:
        return _prog_cache[key]

    groups = _chunk_plan(C)
    nc = bacc.Bacc(None, target_bir_lowering=False)

    xT_d = nc.declare_dram_parameter("xT", [128, KH, C], BF16, isOutput=False)
    w1_d = nc.declare_dram_parameter("w1t", [KI, 128, KH, 128], BF16, isOutput=False)
    w3_d = nc.declare_dram_parameter("w3t", [KI, 128, KH, 128], BF16, isOutput=False)
    w2_d = nc.declare_dram_parameter("w2t", [KH, 128, KI, 128], BF16, isOutput=False)
    out_d = nc.declare_dram_parameter("outT", [KH, 128, C], F32, isOutput=True)

    with tile.TileContext(nc) as tc:
        with (
            tc.tile_pool(name="xg", bufs=1) as xp,
            tc.tile_pool(name="hT", bufs=1) as hp,
            tc.tile_pool(name="wstrip", bufs=2) as wp,
            tc.tile_pool(name="evac", bufs=3) as ep,
            tc.tile_pool(name="ps", bufs=2, space="PSUM") as psp,
        ):
            for g0, gw, chunks in groups:
                xg = xp.tile([128, KH, gw], BF16, tag="xg")
                nc.sync.dma_start(xg[:], xT_d[:, :, g0 : g0 + gw])
                hT = hp.tile([128, KI, gw], BF16, tag="hT")
                # ---- stage 1: hT[it] = silu(w1.T x) * (w3.T x) ----
                for it in range(KI):
                    w1s = wp.tile([128, KH, 128], BF16, tag="w1")
                    w3s = wp.tile([128, KH, 128], BF16, tag="w3")
                    nc.sync.dma_start(w1s[:], w1_d[it])
                    nc.sync.dma_start(w3s[:], w3_d[it])
                    for c0, cw in chunks:
                        ps1 = psp.tile([128, cw], F32, tag="ps1")
                        ps3 = psp.tile([128, cw], F32, tag="ps3")
                        for k in range(KH):
                            nc.tensor.matmul(
                                ps1[:], w1s[:, k, :], xg[:, k, c0 : c0 + cw],
                                start=(k == 0), stop=(k == KH - 1),
                            )
                            nc.tensor.matmul(
                                ps3[:], w3s[:, k, :], xg[:, k, c0 : c0 + cw],
                                start=(k == 0), stop=(k == KH - 1),
                            )
                        st = ep.tile([128, cw], F32, tag="silu")
                        nc.scalar.activation(st[:], ps1[:], AF.Silu)
                        nc.vector.tensor_mul(hT[:, it, c0 : c0 + cw], st[:], ps3[:])
                # ---- stage 2: outT[ht] = w2.T hT ----
                for ht in range(KH):
                    w2s = wp.tile([128, KI, 128], BF16, tag="w2")
                    nc.sync.dma_start(w2s[:], w2_d[ht])
                    for c0, cw in chunks:
                        pso = psp.tile([128, cw], F32, tag="pso")
                        for k in range(KI):
                            nc.tensor.matmul(
                                pso[:], w2s[:, k, :], hT[:, k, c0 : c0 + cw],
                                start=(k == 0), stop=(k == KI - 1),
                            )
                        ot = ep.tile([128, cw], F32, tag="ot")
                        nc.vector.tensor_copy(ot[:], pso[:])
                        nc.sync.dma_start(out_d[ht, :, g0 + c0 : g0 + c0 + cw], ot[:])
    nc.finalize()
    _prog_cache[key] = nc
    return nc


def _route(x: np.ndarray, w_gate: np.ndarray):
    """Replicates the reference router in fp32: softcapped softmax + top-2."""
    logits = x @ w_gate
    logits = (SOFTCAP * np.tanh(logits / SOFTCAP)).astype(np.float32)
    m = logits.max(axis=-1, keepdims=True)
    e = np.exp(logits - m)
    probs = e / e.sum(axis=-1, keepdims=True)
    idx = np.argsort(-probs, axis=-1, kind="stable")[:, :TOPK]
    return probs, idx


def _run(inputs, trace=False, trace_kwargs=None):
    hidden_states = np.asarray(inputs["hidden_states"], dtype=np.float32)
    w_gate = np.asarray(inputs["w_gate"], dtype=np.float32)
    w1 = np.asarray(inputs["w1"], dtype=np.float32)
    w3 = np.asarray(inputs["w3"], dtype=np.float32)
    w2 = np.asarray(inputs["w2"], dtype=np.float32)

    orig_shape = hidden_states.shape
    x = hidden_states.reshape(-1, H)
    T = x.shape[0]

    probs, idx = _route(x, w_gate)
    sel = np.zeros((T, E), dtype=bool)
    sel[np.arange(T), idx[:, 0]] = True
    sel[np.arange(T), idx[:, 1]] = True
    tok_idx = [np.nonzero(sel[:, e])[0] for e in range(E)]
    counts = [len(t) for t in tok_idx]
    C = max(128, -(-max(counts) // 128) * 128)

    nc = _build_program(C)

    x_bf = x.astype(ml_dtypes.bfloat16)
    in_maps = []
    for e in range(E):
        n_e = counts[e]
        xg = np.zeros((C, H), dtype=ml_dtypes.bfloat16)
        xg[:n_e] = x_bf[tok_idx[e]]
        # xT layout [128 p, KH k, C c] with element [p,k,c] = x[c, k*128+p]
        xT = np.ascontiguousarray(xg.T.reshape(KH, 128, C).transpose(1, 0, 2))
        w1t = np.ascontiguousarray(
            w1[e].astype(ml_dtypes.bfloat16).reshape(KH, 128, KI, 128).transpose(2, 1, 0, 3)
        )
        w3t = np.ascontiguousarray(
            w3[e].astype(ml_dtypes.bfloat16).reshape(KH, 128, KI, 128).transpose(2, 1, 0, 3)
        )
        w2t = np.ascontiguousarray(
            w2[e].astype(ml_dtypes.bfloat16).reshape(KI, 128, KH, 128).transpose(2, 1, 0, 3)
        )
        in_maps.append({"xT": xT, "w1t": w1t, "w3t": w3t, "w2t": w2t})

    res = run_bass_kernel_spmd(
        nc, in_maps, core_ids=list(range(E)), trace=trace,
        **(trace_kwargs or {}),
    )

    out = np.zeros((T, H), dtype=np.float32)
    for e in range(E):
        n_e = counts[e]
        outT = res.results[e]["outT"].reshape(H, C)
        wt = probs[tok_idx[e], e].astype(np.float32)
        out[tok_idx[e]] += outT[:, :n_e].T * wt[:, None]
    return out.reshape(orig_shape), res


def kernel(**inputs) -> np.ndarray:
    out, _ = _run(inputs, trace=False)
    return out


# revision 5
# speedup vs baseline: 1.0281x; 1.0281x over previous
"""Grok1-style MoE (E=8 experts, top-2, H=2048, I=4096, T=8192) on 8 trn2 NeuronCores.

Strategy: expert parallelism with host-side routing.
- Host computes the (tiny: ~0.3 GFLOP of ~6.6 TFLOP total) router matmul +
  softcapped softmax + top-2 selection, gathers each expert's tokens, and
  packs per-core inputs. Core e runs expert e's FFN over its ~T*2/E tokens.
- Device kernel per core (bf16 matmuls, fp32 accumulate):
    hT  = silu(w1.T @ xT) * (w3.T @ xT)      # [I, C] in transposed layout
    outT = w2.T @ hT                          # [H, C]
  All operands are laid out on host so every DMA is a contiguous slice and
  every matmul lhsT/rhs is a natural [K=128, M/N] tile.
- Host scatter-adds `probs[t, e] * outT.T` into the full output.
"""

import os
import sys

for _p in ("/opt/trn_rl_repo", "/root/.axon_site/_ro/trn_rl_repo"):
    if os.path.isdir(_p) and _p not in sys.path:
        sys.path.insert(0, _p)

import numpy as np
import ml_dtypes

import concourse.bass as bass  # noqa: F401  (registers types)
import concourse.mybir as mybir
import concourse.tile as tile
from concourse import bacc
from concourse.bass_utils import run_bass_kernel_spmd

BF16 = mybir.dt.bfloat16
F32 = mybir.dt.float32
AF = mybir.ActivationFunctionType

E, TOPK, H, I = 8, 2, 2048, 4096
SOFTCAP = 30.0
KH = H // 128   # 16 k-tiles over H
KI = I // 128   # 32 k-tiles over I
GROUP_MAX = 1152  # max token-columns resident per group (SBUF budget)

_prog_cache: dict = {}


def _chunk_plan(C: int):
    """Split [0, C) into matmul-N chunks (<=512) packed into SBUF groups.

    Chunks narrower than 256 columns are LDWEIGHTS-bound on the PE, so a
    short remainder is rebalanced across the last two chunks instead.
    """
    widths = []
    c = 0
    while c < C:
        w = min(512, C - c)
        widths.append(w)
        c += w
    if len(widths) >= 2 and widths[-1] < 256:
        tot = widths[-2] + widths[-1]
        a = (tot // 2 + 31) // 32 * 32
        widths[-2:] = [a, tot - a]
    chunks = []
    c = 0
    for w in widths:
        chunks.append((c, w))
        c += w
    groups = []  # (g0, gw, [(rel_off, w), ...])
    cur, cur_w = [], 0
    for off, w in chunks:
        if cur and cur_w + w > GROUP_MAX:
            groups.append((cur[0][0], cur_w, [(o - cur[0][0], ww) for o, ww in cur]))
            cur, cur_w = [], 0
        cur.append((off, w))
        cur_w += w
    if cur:
        groups.append((cur[0][0], cur_w, [(o - cur[0][0], ww) for o, ww in cur]))
    return groups


def _build_program(C: int):
    key = C
    if key in _prog_cache:
        return _prog_cache[key]

    groups = _chunk_plan(C)
    nc = bacc.Bacc(None, target_bir_lowering=False)

    xT_d = nc.declare_dram_parameter("xT", [128, KH, C], BF16, isOutput=False)
    w1_d = nc.declare_dram_parameter("w1t", [KI, 128, KH, 128], BF16, isOutput=False)
    w3_d = nc.declare_dram_parameter("w3t", [KI, 128, KH, 128], BF16, isOutput=False)
    w2_d = nc.declare_dram_parameter("w2t", [KH, 128, KI, 128], BF16, isOutput=False)
    out_d = nc.declare_dram_parameter("outT", [KH, 128, C], F32, isOutput=True)

    with tile.TileContext(nc) as tc:
        with (
            tc.tile_pool(name="xg", bufs=1) as xp,
            tc.tile_pool(name="hT", bufs=1) as hp,
            tc.tile_pool(name="wstrip", bufs=2) as wp,
            tc.tile_pool(name="evac", bufs=3) as ep,
            tc.tile_pool(name="ps", bufs=2, space="PSUM") as psp,
        ):
            for g0, gw, chunks in groups:
                # per-k tiles so the first matmul chain only waits on 1/KH
                # of the group's token load
                xgk = []
                for k in range(KH):
                    t = xp.tile([128, gw], BF16, tag=f"xg{k}")
                    nc.sync.dma_start(t[:], xT_d[:, k, g0 : g0 + gw])
                    xgk.append(t)
                hT = hp.tile([128, KI, gw], BF16, tag="hT")
                # ---- stage 1: hT[it] = silu(w1.T x) * (w3.T x) ----
                for it in range(KI):
                    w1s = wp.tile([128, KH, 128], BF16, tag="w1")
                    w3s = wp.tile([128, KH, 128], BF16, tag="w3")
                    nc.sync.dma_start(w1s[:], w1_d[it])
                    nc.sync.dma_start(w3s[:], w3_d[it])
                    for c0, cw in chunks:
                        ps1 = psp.tile([128, cw], F32, tag="ps1")
                        ps3 = psp.tile([128, cw], F32, tag="ps3")
                        for k in range(KH):
                            nc.tensor.matmul(
                                ps1[:], w1s[:, k, :], xgk[k][:, c0 : c0 + cw],
                                start=(k == 0), stop=(k == KH - 1),
                            )
                            nc.tensor.matmul(
                                ps3[:], w3s[:, k, :], xgk[k][:, c0 : c0 + cw],
                                start=(k == 0), stop=(k == KH - 1),
                            )
                        st = ep.tile([128, cw], F32, tag="silu")
                        nc.scalar.activation(st[:], ps1[:], AF.Silu)
                        nc.vector.tensor_mul(hT[:, it, c0 : c0 + cw], st[:], ps3[:])
                # ---- stage 2: outT[ht] = w2.T hT ----
                for ht in range(KH):
                    w2s = wp.tile([128, KI, 128], BF16, tag="w2")
                    nc.sync.dma_start(w2s[:], w2_d[ht])
                    for c0, cw in chunks:
                        pso = psp.tile([128, cw], F32, tag="pso")
                        for k in range(KI):
                            nc.tensor.matmul(
                                pso[:], w2s[:, k, :], hT[:, k, c0 : c0 + cw],
                                start=(k == 0), stop=(k == KI - 1),
                            )
                        ot = ep.tile([128, cw], F32, tag="ot")
                        nc.vector.tensor_copy(ot[:], pso[:])
                        nc.sync.dma_start(out_d[ht, :, g0 + c0 : g0 + c0 + cw], ot[:])
    nc.finalize()
    _prog_cache[key] = nc
    return nc


def _route(x: np.ndarray, w_gate: np.ndarray):
    """Replicates the reference router in fp32: softcapped softmax + top-2."""
    logits = x @ w_gate
    logits = (SOFTCAP * np.tanh(logits / SOFTCAP)).astype(np.float32)
    m = logits.max(axis=-1, keepdims=True)
    e = np.exp(logits - m)
    probs = e / e.sum(axis=-1, keepdims=True)
    idx = np.argsort(-probs, axis=-1, kind="stable")[:, :TOPK]
    return probs, idx


def _run(inputs, trace=False, trace_kwargs=None):
    hidden_states = np.asarray(inputs["hidden_states"], dtype=np.float32)
    w_gate = np.asarray(inputs["w_gate"], dtype=np.float32)
    w1 = np.asarray(inputs["w1"], dtype=np.float32)
    w3 = np.asarray(inputs["w3"], dtype=np.float32)
    w2 = np.asarray(inputs["w2"], dtype=np.float32)

    orig_shape = hidden_states.shape
    x = hidden_states.reshape(-1, H)
    T = x.shape[0]

    probs, idx = _route(x, w_gate)
    sel = np.zeros((T, E), dtype=bool)
    sel[np.arange(T), idx[:, 0]] = True
    sel[np.arange(T), idx[:, 1]] = True
    tok_idx = [np.nonzero(sel[:, e])[0] for e in range(E)]
    counts = [len(t) for t in tok_idx]
    C = max(256, -(-max(counts) // 64) * 64)

    nc = _build_program(C)

    x_bf = x.astype(ml_dtypes.bfloat16)
    in_maps = []
    for e in range(E):
        n_e = counts[e]
        xg = np.zeros((C, H), dtype=ml_dtypes.bfloat16)
        xg[:n_e] = x_bf[tok_idx[e]]
        # xT layout [128 p, KH k, C c] with element [p,k,c] = x[c, k*128+p]
        xT = np.ascontiguousarray(xg.T.reshape(KH, 128, C).transpose(1, 0, 2))
        w1t = np.ascontiguousarray(
            w1[e].astype(ml_dtypes.bfloat16).reshape(KH, 128, KI, 128).transpose(2, 1, 0, 3)
        )
        w3t = np.ascontiguousarray(
            w3[e].astype(ml_dtypes.bfloat16).reshape(KH, 128, KI, 128).transpose(2, 1, 0, 3)
        )
        w2t = np.ascontiguousarray(
            w2[e].astype(ml_dtypes.bfloat16).reshape(KI, 128, KH, 128).transpose(2, 1, 0, 3)
        )
        in_maps.append({"xT": xT, "w1t": w1t, "w3t": w3t, "w2t": w2t})

    res = run_bass_kernel_spmd(
        nc, in_maps, core_ids=list(range(E)), trace=trace,
        **(trace_kwargs or {}),
    )

    out = np.zeros((T, H), dtype=np.float32)
    for e in range(E):
        n_e = counts[e]
        outT = res.results[e]["outT"].reshape(H, C)
        wt = probs[tok_idx[e], e].astype(np.float32)
        out[tok_idx[e]] += outT[:, :n_e].T * wt[:, None]
    return out.reshape(orig_shape), res


def kernel(**inputs) -> np.ndarray:
    out, _ = _run(inputs, trace=False)
    return out


# revision 6
# speedup vs baseline: 1.0341x; 1.0058x over previous
"""Grok1-style MoE (E=8 experts, top-2, H=2048, I=4096, T=8192) on 8 trn2 NeuronCores.

Strategy: expert parallelism with host-side routing.
- Host computes the (tiny: ~0.3 GFLOP of ~6.6 TFLOP total) router matmul +
  softcapped softmax + top-2 selection, gathers each expert's tokens, and
  packs per-core inputs. Core e runs expert e's FFN over its ~T*2/E tokens.
- Device kernel per core (bf16 matmuls, fp32 accumulate):
    hT  = silu(w1.T @ xT) * (w3.T @ xT)      # [I, C] in transposed layout
    outT = w2.T @ hT                          # [H, C]
  All operands are laid out on host so every DMA is a contiguous slice and
  every matmul lhsT/rhs is a natural [K=128, M/N] tile.
- Host scatter-adds `probs[t, e] * outT.T` into the full output.
"""

import os
import sys

for _p in ("/opt/trn_rl_repo", "/root/.axon_site/_ro/trn_rl_repo"):
    if os.path.isdir(_p) and _p not in sys.path:
        sys.path.insert(0, _p)

import numpy as np
import ml_dtypes

import concourse.bass as bass  # noqa: F401  (registers types)
import concourse.mybir as mybir
import concourse.tile as tile
from concourse import bacc
from concourse.bass_utils import run_bass_kernel_spmd

BF16 = mybir.dt.bfloat16
F32 = mybir.dt.float32
AF = mybir.ActivationFunctionType

E, TOPK, H, I = 8, 2, 2048, 4096
SOFTCAP = 30.0
KH = H // 128   # 16 k-tiles over H
KI = I // 128   # 32 k-tiles over I
GROUP_MAX = 1152  # max token-columns resident per group (SBUF budget)

_prog_cache: dict = {}


def _chunk_plan(C: int):
    """Split [0, C) into matmul-N chunks (<=512) packed into SBUF groups.

    Chunks narrower than 256 columns are LDWEIGHTS-bound on the PE, so a
    short remainder is rebalanced across the last two chunks instead.
    """
    widths = []
    c = 0
    while c < C:
        w = min(512, C - c)
        widths.append(w)
        c += w
    if len(widths) >= 2 and widths[-1] < 256:
        tot = widths[-2] + widths[-1]
        a = (tot // 2 + 31) // 32 * 32
        widths[-2:] = [a, tot - a]
    chunks = []
    c = 0
    for w in widths:
        chunks.append((c, w))
        c += w
    groups = []  # (g0, gw, [(rel_off, w), ...])
    cur, cur_w = [], 0
    for off, w in chunks:
        if cur and cur_w + w > GROUP_MAX:
            groups.append((cur[0][0], cur_w, [(o - cur[0][0], ww) for o, ww in cur]))
            cur, cur_w = [], 0
        cur.append((off, w))
        cur_w += w
    if cur:
        groups.append((cur[0][0], cur_w, [(o - cur[0][0], ww) for o, ww in cur]))
    return groups


def _build_program(C: int):
    key = C
    if key in _prog_cache:
        return _prog_cache[key]

    groups = _chunk_plan(C)
    nc = bacc.Bacc(None, target_bir_lowering=False)

    xT_d = nc.declare_dram_parameter("xT", [128, KH, C], BF16, isOutput=False)
    w1_d = nc.declare_dram_parameter("w1t", [KI, 128, KH, 128], BF16, isOutput=False)
    w3_d = nc.declare_dram_parameter("w3t", [KI, 128, KH, 128], BF16, isOutput=False)
    w2_d = nc.declare_dram_parameter("w2t", [KH, 128, KI, 128], BF16, isOutput=False)
    out_d = nc.declare_dram_parameter("outT", [KH, 128, C], F32, isOutput=True)

    with tile.TileContext(nc) as tc:
        with (
            tc.tile_pool(name="xg", bufs=1) as xp,
            tc.tile_pool(name="hT", bufs=1) as hp,
            tc.tile_pool(name="wstrip", bufs=2) as wp,
            tc.tile_pool(name="evac", bufs=3) as ep,
            tc.tile_pool(name="ps", bufs=2, space="PSUM") as psp,
            tc.tile_pool(name="wu", bufs=1) as wup,
            tc.tile_pool(name="wups", bufs=1, space="PSUM") as wupsp,
        ):
            # Warm-up: ~4us of throwaway matmuls so the PE HAM clock-gate
            # reaches 8/8 while the first token/weight DMAs are in flight.
            wu_a = wup.tile([128, 512], BF16, tag="wua")
            nc.vector.memset(wu_a[:], 0.0)
            wu_ps = wupsp.tile([128, 512], F32, tag="wups")
            for _ in range(20):
                nc.tensor.matmul(wu_ps[:], wu_a[:, :128], wu_a[:], start=True, stop=True)

            for gi, (g0, gw, chunks) in enumerate(groups):
                # First group: the opening matmul chain needs w-strips for
                # it=0/1, so queue those DMAs ahead of the bulk token load.
                pre_w = {}
                if gi == 0:
                    for it in range(2):
                        w1s = wp.tile([128, KH, 128], BF16, tag="w1")
                        w3s = wp.tile([128, KH, 128], BF16, tag="w3")
                        nc.sync.dma_start(w1s[:], w1_d[it])
                        nc.sync.dma_start(w3s[:], w3_d[it])
                        pre_w[it] = (w1s, w3s)
                # per-k tiles so the first matmul chain only waits on 1/KH
                # of the group's token load
                xgk = []
                for k in range(KH):
                    t = xp.tile([128, gw], BF16, tag=f"xg{k}")
                    nc.sync.dma_start(t[:], xT_d[:, k, g0 : g0 + gw])
                    xgk.append(t)
                hT = hp.tile([128, KI, gw], BF16, tag="hT")
                # ---- stage 1: hT[it] = silu(w1.T x) * (w3.T x) ----
                for it in range(KI):
                    if it in pre_w:
                        w1s, w3s = pre_w[it]
                    else:
                        w1s = wp.tile([128, KH, 128], BF16, tag="w1")
                        w3s = wp.tile([128, KH, 128], BF16, tag="w3")
                        nc.sync.dma_start(w1s[:], w1_d[it])
                        nc.sync.dma_start(w3s[:], w3_d[it])
                    for c0, cw in chunks:
                        ps1 = psp.tile([128, cw], F32, tag="ps1")
                        ps3 = psp.tile([128, cw], F32, tag="ps3")
                        for k in range(KH):
                            nc.tensor.matmul(
                                ps1[:], w1s[:, k, :], xgk[k][:, c0 : c0 + cw],
                                start=(k == 0), stop=(k == KH - 1),
                            )
                            nc.tensor.matmul(
                                ps3[:], w3s[:, k, :], xgk[k][:, c0 : c0 + cw],
                                start=(k == 0), stop=(k == KH - 1),
                            )
                        st = ep.tile([128, cw], F32, tag="silu")
                        nc.scalar.activation(st[:], ps1[:], AF.Silu)
                        nc.vector.tensor_mul(hT[:, it, c0 : c0 + cw], st[:], ps3[:])
                # ---- stage 2: outT[ht] = w2.T hT ----
                for ht in range(KH):
                    w2s = wp.tile([128, KI, 128], BF16, tag="w2")
                    nc.sync.dma_start(w2s[:], w2_d[ht])
                    for c0, cw in chunks:
                        pso = psp.tile([128, cw], F32, tag="pso")
                        for k in range(KI):
                            nc.tensor.matmul(
                                pso[:], w2s[:, k, :], hT[:, k, c0 : c0 + cw],
                                start=(k == 0), stop=(k == KI - 1),
                            )
                        ot = ep.tile([128, cw], F32, tag="ot")
                        nc.vector.tensor_copy(ot[:], pso[:])
                        nc.sync.dma_start(out_d[ht, :, g0 + c0 : g0 + c0 + cw], ot[:])
    nc.finalize()
    _prog_cache[key] = nc
    return nc


def _route(x: np.ndarray, w_gate: np.ndarray):
    """Replicates the reference router in fp32: softcapped softmax + top-2."""
    logits = x @ w_gate
    logits = (SOFTCAP * np.tanh(logits / SOFTCAP)).astype(np.float32)
    m = logits.max(axis=-1, keepdims=True)
    e = np.exp(logits - m)
    probs = e / e.sum(axis=-1, keepdims=True)
    idx = np.argsort(-probs, axis=-1, kind="stable")[:, :TOPK]
    return probs, idx


def _run(inputs, trace=False, trace_kwargs=None):
    hidden_states = np.asarray(inputs["hidden_states"], dtype=np.float32)
    w_gate = np.asarray(inputs["w_gate"], dtype=np.float32)
    w1 = np.asarray(inputs["w1"], dtype=np.float32)
    w3 = np.asarray(inputs["w3"], dtype=np.float32)
    w2 = np.asarray(inputs["w2"], dtype=np.float32)

    orig_shape = hidden_states.shape
    x = hidden_states.reshape(-1, H)
    T = x.shape[0]

    probs, idx = _route(x, w_gate)
    sel = np.zeros((T, E), dtype=bool)
    sel[np.arange(T), idx[:, 0]] = True
    sel[np.arange(T), idx[:, 1]] = True
    tok_idx = [np.nonzero(sel[:, e])[0] for e in range(E)]
    counts = [len(t) for t in tok_idx]
    C = max(256, -(-max(counts) // 64) * 64)

    nc = _build_program(C)

    x_bf = x.astype(ml_dtypes.bfloat16)
    in_maps = []
    for e in range(E):
        n_e = counts[e]
        xg = np.zeros((C, H), dtype=ml_dtypes.bfloat16)
        xg[:n_e] = x_bf[tok_idx[e]]
        # xT layout [128 p, KH k, C c] with element [p,k,c] = x[c, k*128+p]
        xT = np.ascontiguousarray(xg.T.reshape(KH, 128, C).transpose(1, 0, 2))
        w1t = np.ascontiguousarray(
            w1[e].astype(ml_dtypes.bfloat16).reshape(KH, 128, KI, 128).transpose(2, 1, 0, 3)
        )
        w3t = np.ascontiguousarray(
            w3[e].astype(ml_dtypes.bfloat16).reshape(KH, 128, KI, 128).transpose(2, 1, 0, 3)
        )
        w2t = np.ascontiguousarray(
            w2[e].astype(ml_dtypes.bfloat16).reshape(KI, 128, KH, 128).transpose(2, 1, 0, 3)
        )
        in_maps.append({"xT": xT, "w1t": w1t, "w3t": w3t, "w2t": w2t})

    res = run_bass_kernel_spmd(
        nc, in_maps, core_ids=list(range(E)), trace=trace,
        **(trace_kwargs or {}),
    )

    out = np.zeros((T, H), dtype=np.float32)
    for e in range(E):
        n_e = counts[e]
        outT = res.results[e]["outT"].reshape(H, C)
        wt = probs[tok_idx[e], e].astype(np.float32)
        out[tok_idx[e]] += outT[:, :n_e].T * wt[:, None]
    return out.reshape(orig_shape), res


def kernel(**inputs) -> np.ndarray:
    out, _ = _run(inputs, trace=False)
    return out
